# revision 7
# baseline (speedup 1.0000x reference)
"""Trainium2 Bass kernel for BarycentricCoordinates (retrieval_knn).

Computes, per (v, r, a) problem: nearest-neighbor ordering of 8 projected
points vs a template vertex, barycentric weights for every candidate
(second, third) vertex pair, Delaunay empty-circumcircle filter, and
selection of the min-inf-norm-squared-weight pair.

Algorithm works in ORIGINAL k0-index space (no argsort/gather on device):
 - closest point c = argmin_k d2[k] via min-reduce + one-hot equality
 - all 64 ordered pairs (i,j) are candidates; pairs with i==j, i==c, j==c
   are masked. The Delaunay orientation test (det >= 0 for all k) keeps at
   most one ordering of each unordered pair, matching the reference's
   argsort-space tie-break (validated exactly vs reference on CPU).
 - fallback (all candidates invalid): weights=0, indices=[c, o1, o1].

Layout: partitions = 128 v's per tile. Pair/det tensors use (ij, r)-major
free layout so every operand view fits the 3-free-dim ISA AP limit (leading
broadcast dims merge into one 0-step dim). The det-stage cross terms
u1/u2/u3 depend only on (i,j,k) — hoisted out of the ra loop, computed once
per v-tile. Embarrassingly parallel over V across 8 cores.
"""

import sys

sys.path.insert(0, "/opt/trn_rl_repo")

import numpy as np

import concourse.bass as bass
import concourse.bacc as bacc
import concourse.mybir as mybir
from concourse.tile import TileContext

F32 = mybir.dt.float32
I32 = mybir.dt.int32
OP = mybir.AluOpType
AF = mybir.ActivationFunctionType
AX = mybir.AxisListType

BIG = 2.0e38
N_CORES = 8
V_TOTAL = 5000
R, A, K0 = 5, 8, 8
RA = R * A            # 40
VS = V_TOTAL // N_CORES   # 625 per core
P = 128
VSP = 640             # padded to 5 full partition tiles
RC = 10               # ra's per chunk
N_VT = VSP // P       # 5
N_RCH = RA // RC      # 4
K2 = 64               # pairs (i,j)
K3 = 512              # triples (i,j,k)


def build_nc(vsp=VSP, rc=RC, ra=RA):
    nc = bacc.Bacc("TRN2", target_bir_lowering=False)
    n_vt = vsp // P
    n_rch = ra // rc

    px_d = nc.dram_tensor("px", (vsp, K0), F32, kind="ExternalInput")
    py_d = nc.dram_tensor("py", (vsp, K0), F32, kind="ExternalInput")
    tmpl_d = nc.dram_tensor("tmpl", (2, ra), F32, kind="ExternalInput")
    eyeE_d = nc.dram_tensor("eyeE", (1, K3), F32, kind="ExternalInput")
    neq_d = nc.dram_tensor("neq", (1, K2), F32, kind="ExternalInput")
    iota8_d = nc.dram_tensor("iota8", (1, K0), F32, kind="ExternalInput")
    iotaI_d = nc.dram_tensor("iotaI", (1, K2), F32, kind="ExternalInput")
    iotaJ_d = nc.dram_tensor("iotaJ", (1, K2), F32, kind="ExternalInput")
    iota64ij_d = nc.dram_tensor("iota64ij", (1, K2 * rc), F32, kind="ExternalInput")
    outw_d = nc.dram_tensor("outw", (vsp, ra, 3), F32, kind="ExternalOutput")
    outi_d = nc.dram_tensor("outi", (vsp, ra, 3), F32, kind="ExternalOutput")

    with TileContext(nc) as tc:
        VE = nc.vector
        GP = nc.gpsimd
        SC = nc.scalar
        S = rc * K3           # det stage free size (ij, r, k)
        PP = rc * K2          # pair stage free size (ij, r)
        RK = rc * K0          # (r, k)

        with (
            tc.tile_pool(name="const", bufs=1) as cpool,
            tc.tile_pool(name="vt", bufs=2) as vpool,
            tc.tile_pool(name="det", bufs=1) as spool,
            tc.tile_pool(name="pair", bufs=2) as ppool,
            tc.tile_pool(name="rk", bufs=2) as rkpool,
            tc.tile_pool(name="small", bufs=2) as opool,
        ):
            # ---- constants, materialized once across all partitions ----
            TX = cpool.tile([P, ra], F32, tag="TX")
            TY = cpool.tile([P, ra], F32, tag="TY")
            E = cpool.tile([P, K3], F32, tag="E")
            NEQ = cpool.tile([P, K2], F32, tag="NEQ")
            IOTA8 = cpool.tile([P, K0], F32, tag="IOTA8")
            IOTAI = cpool.tile([P, K2], F32, tag="IOTAI")
            IOTAJ = cpool.tile([P, K2], F32, tag="IOTAJ")
            IOTA64IJ = cpool.tile([P, K2 * rc], F32, tag="IOTA64IJ")
            nc.sync.dma_start(TX, tmpl_d[0:1, :].to_broadcast((P, ra)))
            nc.sync.dma_start(TY, tmpl_d[1:2, :].to_broadcast((P, ra)))
            nc.sync.dma_start(E, eyeE_d[0:1, :].to_broadcast((P, K3)))
            nc.sync.dma_start(NEQ, neq_d[0:1, :].to_broadcast((P, K2)))
            nc.sync.dma_start(IOTA8, iota8_d[0:1, :].to_broadcast((P, K0)))
            nc.sync.dma_start(IOTAI, iotaI_d[0:1, :].to_broadcast((P, K2)))
            nc.sync.dma_start(IOTAJ, iotaJ_d[0:1, :].to_broadcast((P, K2)))
            nc.sync.dma_start(IOTA64IJ, iota64ij_d[0:1, :].to_broadcast((P, K2 * rc)))

            def bcv(ap, shape):
                return ap.to_broadcast(shape)

            for vt in range(n_vt):
                v0_, v1_ = vt * P, (vt + 1) * P
                px = vpool.tile([P, K0], F32, tag="px")
                py = vpool.tile([P, K0], F32, tag="py")
                nc.sync.dma_start(px, px_d[v0_:v1_, :])
                nc.sync.dma_start(py, py_d[v0_:v1_, :])

                s_ = vpool.tile([P, K0], F32, tag="s")
                t8 = vpool.tile([P, K0], F32, tag="t8")
                VE.tensor_tensor(out=s_, in0=px, in1=px, op=OP.mult)
                VE.tensor_tensor(out=t8, in0=py, in1=py, op=OP.mult)
                VE.tensor_tensor(out=s_, in0=s_, in1=t8, op=OP.add)

                # ---- b-tensors (i,k) and hoisted det cross terms (i,j,k) ----
                # b* = p[i] - p[k]; u1 = by_i*bs_j - bs_i*by_j  (d == b at (j,k)),
                # u2 = bx_i*bs_j - bs_i*bx_j, u3 = bx_i*by_j - by_i*bx_j
                bx = vpool.tile([P, K2], F32, tag="bx")
                by = vpool.tile([P, K2], F32, tag="by")
                bs = vpool.tile([P, K2], F32, tag="bs")
                bxv = bx.rearrange("p (i k) -> p i k", k=K0)
                byv = by.rearrange("p (i k) -> p i k", k=K0)
                bsv = bs.rearrange("p (i k) -> p i k", k=K0)
                VE.tensor_tensor(out=bxv, in0=bcv(px.unsqueeze(2), (P, K0, K0)),
                                 in1=bcv(px.unsqueeze(1), (P, K0, K0)), op=OP.subtract)
                VE.tensor_tensor(out=byv, in0=bcv(py.unsqueeze(2), (P, K0, K0)),
                                 in1=bcv(py.unsqueeze(1), (P, K0, K0)), op=OP.subtract)
                VE.tensor_tensor(out=bsv, in0=bcv(s_.unsqueeze(2), (P, K0, K0)),
                                 in1=bcv(s_.unsqueeze(1), (P, K0, K0)), op=OP.subtract)

                def Bi(t):   # (i,k) -> (i,j,k), broadcast j
                    return bcv(t.rearrange("p (i k) -> p i k", k=K0).unsqueeze(2),
                               (P, K0, K0, K0))

                def Bj(t):   # (i,k) -> (i,j,k) read as (j,k), broadcast i
                    return bcv(t.rearrange("p (j k) -> p j k", k=K0).unsqueeze(1),
                               (P, K0, K0, K0))

                U1 = vpool.tile([P, K3], F32, tag="U1")
                U2 = vpool.tile([P, K3], F32, tag="U2")
                U3 = vpool.tile([P, K3], F32, tag="U3")
                uA = vpool.tile([P, K3], F32, tag="uA")
                U1v = U1.rearrange("p (i j k) -> p i j k", j=K0, k=K0)
                U2v = U2.rearrange("p (i j k) -> p i j k", j=K0, k=K0)
                U3v = U3.rearrange("p (i j k) -> p i j k", j=K0, k=K0)
                uAv = uA.rearrange("p (i j k) -> p i j k", j=K0, k=K0)
                VE.tensor_tensor(out=U1v, in0=Bi(by), in1=Bj(bs), op=OP.mult)
                GP.tensor_tensor(out=uAv, in0=Bi(bs), in1=Bj(by), op=OP.mult)
                VE.tensor_tensor(out=U1, in0=U1, in1=uA, op=OP.subtract)
                GP.tensor_tensor(out=U2v, in0=Bi(bx), in1=Bj(bs), op=OP.mult)
                VE.tensor_tensor(out=uAv, in0=Bi(bs), in1=Bj(bx), op=OP.mult)
                GP.tensor_tensor(out=U2, in0=U2, in1=uA, op=OP.subtract)
                VE.tensor_tensor(out=U3v, in0=Bi(bx), in1=Bj(by), op=OP.mult)
                GP.tensor_tensor(out=uAv, in0=Bi(by), in1=Bj(bx), op=OP.mult)
                VE.tensor_tensor(out=U3, in0=U3, in1=uA, op=OP.subtract)

                for rchunk in range(n_rch):
                    r0 = rchunk * rc
                    r1 = r0 + rc
                    # ---------- stage A: distances, closest; (r, k) layout ----------
                    d2 = rkpool.tile([P, RK], F32, tag="d2")
                    tdx = rkpool.tile([P, RK], F32, tag="tdx")
                    tdy = rkpool.tile([P, RK], F32, tag="tdy")
                    d2v = d2.rearrange("p (r k) -> p r k", k=K0)
                    tdxv = tdx.rearrange("p (r k) -> p r k", k=K0)
                    tdyv = tdy.rearrange("p (r k) -> p r k", k=K0)
                    px_rk = bcv(px.unsqueeze(1), (P, rc, K0))
                    py_rk = bcv(py.unsqueeze(1), (P, rc, K0))
                    s_rk = bcv(s_.unsqueeze(1), (P, rc, K0))
                    tx_rk = bcv(TX[:, r0:r1].unsqueeze(2), (P, rc, K0))
                    ty_rk = bcv(TY[:, r0:r1].unsqueeze(2), (P, rc, K0))
                    VE.tensor_tensor(out=tdxv, in0=px_rk, in1=tx_rk, op=OP.subtract)
                    GP.tensor_tensor(out=tdyv, in0=py_rk, in1=ty_rk, op=OP.subtract)
                    VE.tensor_tensor(out=tdx, in0=tdx, in1=tdx, op=OP.mult)
                    GP.tensor_tensor(out=tdy, in0=tdy, in1=tdy, op=OP.mult)
                    VE.tensor_tensor(out=d2, in0=tdx, in1=tdy, op=OP.add)

                    dmin = opool.tile([P, rc], F32, tag="dmin")
                    VE.tensor_reduce(out=dmin, in_=d2v, axis=AX.X, op=OP.min)
                    dmin_rk = bcv(dmin.unsqueeze(2), (P, rc, K0))
                    m0 = rkpool.tile([P, RK], F32, tag="m0")
                    m0v = m0.rearrange("p (r k) -> p r k", k=K0)
                    VE.tensor_tensor(out=m0v, in0=d2v, in1=dmin_rk, op=OP.is_equal)
                    dgt = rkpool.tile([P, RK], F32, tag="dgt")
                    dgtv = dgt.rearrange("p (r k) -> p r k", k=K0)
                    VE.tensor_tensor(out=dgtv, in0=d2v, in1=dmin_rk, op=OP.is_gt)

                    # gathers: cx, cy, sc; c_f; o1_f
                    tA = rkpool.tile([P, RK], F32, tag="tA")
                    tAv = tA.rearrange("p (r k) -> p r k", k=K0)
                    cx = opool.tile([P, rc], F32, tag="cx")
                    cy = opool.tile([P, rc], F32, tag="cy")
                    sc_ = opool.tile([P, rc], F32, tag="sc_")
                    c_f = opool.tile([P, rc], F32, tag="c_f")
                    o1_f = opool.tile([P, rc], F32, tag="o1_f")
                    VE.tensor_tensor(out=tAv, in0=m0v, in1=px_rk, op=OP.mult)
                    VE.tensor_reduce(out=cx, in_=tAv, axis=AX.X, op=OP.add)
                    VE.tensor_tensor(out=tAv, in0=m0v, in1=py_rk, op=OP.mult)
                    VE.tensor_reduce(out=cy, in_=tAv, axis=AX.X, op=OP.add)
                    VE.tensor_tensor(out=tAv, in0=m0v, in1=s_rk, op=OP.mult)
                    VE.tensor_reduce(out=sc_, in_=tAv, axis=AX.X, op=OP.add)
                    i8_rk = bcv(IOTA8.unsqueeze(1), (P, rc, K0))
                    GP.tensor_tensor(out=tAv, in0=m0v, in1=i8_rk, op=OP.mult)
                    VE.tensor_reduce(out=c_f, in_=tAv, axis=AX.X, op=OP.add)
                    d2b = rkpool.tile([P, RK], F32, tag="d2b")
                    VE.scalar_tensor_tensor(out=d2b, in0=m0, scalar=BIG, in1=d2,
                                            op0=OP.mult, op1=OP.add)
                    dmin2 = opool.tile([P, rc], F32, tag="dmin2")
                    d2bv = d2b.rearrange("p (r k) -> p r k", k=K0)
                    VE.tensor_reduce(out=dmin2, in_=d2bv, axis=AX.X, op=OP.min)
                    dmin2_rk = bcv(dmin2.unsqueeze(2), (P, rc, K0))
                    VE.tensor_tensor(out=tAv, in0=d2bv, in1=dmin2_rk, op=OP.is_equal)
                    GP.tensor_tensor(out=tAv, in0=tAv, in1=i8_rk, op=OP.mult)
                    VE.tensor_reduce(out=o1_f, in_=tAv, axis=AX.X, op=OP.add)

                    # ---------- stage B: v0, dots; (r, k) layout ----------
                    v0x = rkpool.tile([P, RK], F32, tag="v0x")
                    v0y = rkpool.tile([P, RK], F32, tag="v0y")
                    ass = rkpool.tile([P, RK], F32, tag="ass")
                    d00 = rkpool.tile([P, RK], F32, tag="d00")
                    d02 = rkpool.tile([P, RK], F32, tag="d02")
                    v0xv = v0x.rearrange("p (r k) -> p r k", k=K0)
                    v0yv = v0y.rearrange("p (r k) -> p r k", k=K0)
                    assv = ass.rearrange("p (r k) -> p r k", k=K0)
                    d02v = d02.rearrange("p (r k) -> p r k", k=K0)
                    cx_rk = bcv(cx.unsqueeze(2), (P, rc, K0))
                    cy_rk = bcv(cy.unsqueeze(2), (P, rc, K0))
                    sc_rk = bcv(sc_.unsqueeze(2), (P, rc, K0))
                    VE.tensor_tensor(out=v0xv, in0=px_rk, in1=cx_rk, op=OP.subtract)
                    GP.tensor_tensor(out=v0yv, in0=py_rk, in1=cy_rk, op=OP.subtract)
                    GP.tensor_tensor(out=assv, in0=s_rk, in1=sc_rk, op=OP.subtract)
                    v2x = opool.tile([P, rc], F32, tag="v2x")
                    v2y = opool.tile([P, rc], F32, tag="v2y")
                    VE.tensor_tensor(out=v2x, in0=TX[:, r0:r1], in1=cx, op=OP.subtract)
                    VE.tensor_tensor(out=v2y, in0=TY[:, r0:r1], in1=cy, op=OP.subtract)
                    tB = rkpool.tile([P, RK], F32, tag="tB")
                    tBv = tB.rearrange("p (r k) -> p r k", k=K0)
                    VE.tensor_tensor(out=d00, in0=v0x, in1=v0x, op=OP.mult)
                    GP.tensor_tensor(out=tB, in0=v0y, in1=v0y, op=OP.mult)
                    VE.tensor_tensor(out=d00, in0=d00, in1=tB, op=OP.add)
                    v2x_rk = bcv(v2x.unsqueeze(2), (P, rc, K0))
                    v2y_rk = bcv(v2y.unsqueeze(2), (P, rc, K0))
                    VE.tensor_tensor(out=d02v, in0=v0xv, in1=v2x_rk, op=OP.mult)
                    GP.tensor_tensor(out=tBv, in0=v0yv, in1=v2y_rk, op=OP.mult)
                    VE.tensor_tensor(out=d02, in0=d02, in1=tB, op=OP.add)

                    # ---------- stage C: delaunay dets; (ij, r, k) layout ----------
                    # det' = v0x(r,k)*U1(ij,k) - v0y(r,k)*U2(ij,k) + ass(r,k)*U3(ij,k)
                    def Uv(t):   # (ij*k) -> (ij, r, k), broadcast r
                        return bcv(t.rearrange("p (ij k) -> p ij k", k=K0).unsqueeze(2),
                                   (P, K2, rc, K0))

                    def Kv(t2):  # (r,k) -> (ij, r, k), broadcast ij
                        return bcv(t2.rearrange("p (r k) -> p r k", k=K0).unsqueeze(1),
                                   (P, K2, rc, K0))

                    sA = spool.tile([P, S], F32, tag="sA")
                    sB = spool.tile([P, S], F32, tag="sB")
                    sAv = sA.rearrange("p (ij r k) -> p ij r k", r=rc, k=K0)
                    sBv = sB.rearrange("p (ij r k) -> p ij r k", r=rc, k=K0)
                    VE.tensor_tensor(out=sAv, in0=Kv(v0x), in1=Uv(U1), op=OP.mult)
                    GP.tensor_tensor(out=sBv, in0=Kv(v0y), in1=Uv(U2), op=OP.mult)
                    VE.tensor_tensor(out=sA, in0=sA, in1=sB, op=OP.subtract)
                    GP.tensor_tensor(out=sBv, in0=Kv(ass), in1=Uv(U3), op=OP.mult)
                    VE.tensor_tensor(out=sA, in0=sA, in1=sB, op=OP.add)
                    VE.tensor_tensor(out=sAv, in0=sAv, in1=Uv(E), op=OP.max)
                    mindet = ppool.tile([P, PP], F32, tag="mindet")
                    mindetv = mindet.rearrange("p (ij r) -> p ij r", r=rc)
                    VE.tensor_reduce(out=mindetv, in_=sAv, axis=AX.X, op=OP.min)

                    # ---------- stage D: pair weights; (ij, r) layout ----------
                    # (r,k) tiles read as (i, j, r): i from the k slot (stride 1),
                    # j broadcast, r stride K0 — or j from k slot, i broadcast.
                    def XI(t2):
                        v = t2.rearrange("p (r k) -> p k r", k=K0)
                        return bcv(v.unsqueeze(2), (P, K0, K0, rc))

                    def XJ(t2):
                        v = t2.rearrange("p (r k) -> p k r", k=K0)
                        return bcv(v.unsqueeze(1), (P, K0, K0, rc))

                    dot01 = ppool.tile([P, PP], F32, tag="dot01")
                    pA = ppool.tile([P, PP], F32, tag="pA")
                    pB = ppool.tile([P, PP], F32, tag="pB")
                    w1t = ppool.tile([P, PP], F32, tag="w1t")
                    w2t = ppool.tile([P, PP], F32, tag="w2t")
                    w0t = ppool.tile([P, PP], F32, tag="w0t")
                    inv = ppool.tile([P, PP], F32, tag="inv")
                    dot01v = dot01.rearrange("p (i j r) -> p i j r", j=K0, r=rc)
                    pAv = pA.rearrange("p (i j r) -> p i j r", j=K0, r=rc)
                    pBv = pB.rearrange("p (i j r) -> p i j r", j=K0, r=rc)

                    VE.tensor_tensor(out=dot01v, in0=XI(v0x), in1=XJ(v0x), op=OP.mult)
                    GP.tensor_tensor(out=pAv, in0=XI(v0y), in1=XJ(v0y), op=OP.mult)
                    VE.tensor_tensor(out=dot01, in0=dot01, in1=pA, op=OP.add)
                    GP.tensor_tensor(out=pAv, in0=XI(d00), in1=XJ(d00), op=OP.mult)
                    VE.tensor_tensor(out=pB, in0=dot01, in1=dot01, op=OP.mult)
                    VE.tensor_tensor(out=pA, in0=pA, in1=pB, op=OP.subtract)  # denom
                    VE.reciprocal(out=inv, in_=pA)
                    GP.tensor_tensor(out=pB, in0=pA, in1=inv, op=OP.mult)
                    SC.activation(out=pB, in_=pB, func=AF.Copy, bias=2.0, scale=-1.0)
                    VE.tensor_tensor(out=inv, in0=inv, in1=pB, op=OP.mult)

                    VE.tensor_tensor(out=pAv, in0=XJ(d00), in1=XI(d02), op=OP.mult)
                    GP.tensor_tensor(out=pBv, in0=dot01v, in1=XJ(d02), op=OP.mult)
                    VE.tensor_tensor(out=w2t, in0=pA, in1=pB, op=OP.subtract)
                    VE.tensor_tensor(out=w2t, in0=w2t, in1=inv, op=OP.mult)
                    GP.tensor_tensor(out=pAv, in0=XI(d00), in1=XJ(d02), op=OP.mult)
                    VE.tensor_tensor(out=pBv, in0=dot01v, in1=XI(d02), op=OP.mult)
                    VE.tensor_tensor(out=w1t, in0=pA, in1=pB, op=OP.subtract)
                    VE.tensor_tensor(out=w1t, in0=w1t, in1=inv, op=OP.mult)
                    VE.tensor_tensor(out=pA, in0=w2t, in1=w1t, op=OP.add)
                    SC.activation(out=w0t, in_=pA, func=AF.Copy, bias=1.0, scale=-1.0)

                    wm = ppool.tile([P, PP], F32, tag="wm")
                    VE.tensor_tensor(out=wm, in0=w1t, in1=w2t, op=OP.min)
                    VE.tensor_tensor(out=wm, in0=wm, in1=w0t, op=OP.min)
                    VE.tensor_scalar(out=wm, in0=wm, scalar1=0.0, scalar2=None,
                                     op0=OP.is_gt)
                    sq = ppool.tile([P, PP], F32, tag="sq")
                    sr = ppool.tile([P, PP], F32, tag="sr")
                    SC.activation(out=sr, in_=w0t, func=AF.Square)
                    SC.activation(out=sq, in_=w1t, func=AF.Square)
                    VE.tensor_tensor(out=sr, in0=sr, in1=sq, op=OP.max)
                    SC.activation(out=sq, in_=w2t, func=AF.Square)
                    VE.tensor_tensor(out=sr, in0=sr, in1=sq, op=OP.max)

                    valid = ppool.tile([P, PP], F32, tag="valid")
                    validv = valid.rearrange("p (i j r) -> p i j r", j=K0, r=rc)
                    VE.tensor_scalar(out=valid, in0=mindet, scalar1=0.0, scalar2=None,
                                     op0=OP.is_ge)
                    VE.tensor_tensor(out=valid, in0=valid, in1=wm, op=OP.mult)
                    GP.tensor_tensor(out=validv, in0=validv, in1=XI(dgt), op=OP.mult)
                    GP.tensor_tensor(out=validv, in0=validv, in1=XJ(dgt), op=OP.mult)
                    neq_v = bcv(NEQ.rearrange("p (i j) -> p i j", j=K0).unsqueeze(3),
                                (P, K0, K0, rc))
                    VE.tensor_tensor(out=validv, in0=validv, in1=neq_v, op=OP.mult)

                    # score + argmin (first-index tie-break via iota64ij)
                    score = ppool.tile([P, PP], F32, tag="score")
                    VE.memset(score, BIG)
                    VE.copy_predicated(out=score, mask=valid.bitcast(I32), data=sr)
                    score_rij = score.rearrange("p (ij r) -> p r ij", r=rc)
                    smin = opool.tile([P, rc], F32, tag="smin")
                    VE.tensor_reduce(out=smin, in_=score_rij, axis=AX.X, op=OP.min)
                    smin_ij = bcv(smin.unsqueeze(1), (P, K2, rc))
                    eqm = ppool.tile([P, PP], F32, tag="eqm")
                    eqmv = eqm.rearrange("p (ij r) -> p ij r", r=rc)
                    scorev = score.rearrange("p (ij r) -> p ij r", r=rc)
                    VE.tensor_tensor(out=eqmv, in0=scorev, in1=smin_ij, op=OP.is_equal)
                    pidt = ppool.tile([P, PP], F32, tag="pidt")
                    VE.memset(pidt, BIG)
                    VE.copy_predicated(out=pidt, mask=eqm.bitcast(I32), data=IOTA64IJ)
                    pidx = opool.tile([P, rc], F32, tag="pidx")
                    pidt_rij = pidt.rearrange("p (ij r) -> p r ij", r=rc)
                    VE.tensor_reduce(out=pidx, in_=pidt_rij, axis=AX.X, op=OP.min)
                    pidx_ij = bcv(pidx.unsqueeze(1), (P, K2, rc))
                    oh = ppool.tile([P, PP], F32, tag="oh")
                    ohv_ = oh.rearrange("p (ij r) -> p ij r", r=rc)
                    i64v = IOTA64IJ.rearrange("p (ij r) -> p ij r", r=rc)
                    VE.tensor_tensor(out=ohv_, in0=i64v, in1=pidx_ij, op=OP.is_equal)
                    ohm = ppool.tile([P, PP], F32, tag="ohm")
                    VE.tensor_tensor(out=ohm, in0=oh, in1=valid, op=OP.mult)

                    # gather weights (NaN-safe: predicated copy, then sum over ij)
                    g0 = ppool.tile([P, PP], F32, tag="g0")
                    g0_rij = g0.rearrange("p (ij r) -> p r ij", r=rc)
                    w0sel = opool.tile([P, rc], F32, tag="w0sel")
                    w1sel = opool.tile([P, rc], F32, tag="w1sel")
                    w2sel = opool.tile([P, rc], F32, tag="w2sel")
                    VE.memset(g0, 0.0)
                    VE.copy_predicated(out=g0, mask=ohm.bitcast(I32), data=w0t)
                    VE.tensor_reduce(out=w0sel, in_=g0_rij, axis=AX.X, op=OP.add)
                    VE.memset(g0, 0.0)
                    VE.copy_predicated(out=g0, mask=ohm.bitcast(I32), data=w2t)
                    VE.tensor_reduce(out=w2sel, in_=g0_rij, axis=AX.X, op=OP.add)
                    VE.memset(g0, 0.0)
                    VE.copy_predicated(out=g0, mask=ohm.bitcast(I32), data=w1t)
                    VE.tensor_reduce(out=w1sel, in_=g0_rij, axis=AX.X, op=OP.add)

                    # gather pair indices
                    i_f = opool.tile([P, rc], F32, tag="i_f")
                    j_f = opool.tile([P, rc], F32, tag="j_f")
                    iI_v = bcv(IOTAI.rearrange("p (i j) -> p i j", j=K0).unsqueeze(3),
                               (P, K0, K0, rc))
                    iJ_v = bcv(IOTAJ.rearrange("p (i j) -> p i j", j=K0).unsqueeze(3),
                               (P, K0, K0, rc))
                    ohv4 = oh.rearrange("p (i j r) -> p i j r", j=K0, r=rc)
                    g0v4 = g0.rearrange("p (i j r) -> p i j r", j=K0, r=rc)
                    GP.tensor_tensor(out=g0v4, in0=ohv4, in1=iI_v, op=OP.mult)
                    VE.tensor_reduce(out=i_f, in_=g0_rij, axis=AX.X, op=OP.add)
                    GP.tensor_tensor(out=g0v4, in0=ohv4, in1=iJ_v, op=OP.mult)
                    VE.tensor_reduce(out=j_f, in_=g0_rij, axis=AX.X, op=OP.add)

                    # fallback: all candidates invalid
                    fb = opool.tile([P, rc], F32, tag="fb")
                    VE.tensor_scalar(out=fb, in0=smin, scalar1=1.0e38, scalar2=None,
                                     op0=OP.is_ge)
                    VE.copy_predicated(out=i_f, mask=fb.bitcast(I32), data=o1_f)
                    VE.copy_predicated(out=j_f, mask=fb.bitcast(I32), data=o1_f)

                    # ---------- outputs ----------
                    wout = opool.tile([P, rc * 3], F32, tag="wout")
                    iout = opool.tile([P, rc * 3], F32, tag="iout")
                    woutv = wout.rearrange("p (r c) -> p r c", c=3)
                    ioutv = iout.rearrange("p (r c) -> p r c", c=3)
                    SC.copy(out=woutv[:, :, 0], in_=w0sel)
                    SC.copy(out=woutv[:, :, 1], in_=w2sel)
                    SC.copy(out=woutv[:, :, 2], in_=w1sel)
                    SC.copy(out=ioutv[:, :, 0], in_=c_f)
                    SC.copy(out=ioutv[:, :, 1], in_=i_f)
                    SC.copy(out=ioutv[:, :, 2], in_=j_f)
                    nc.sync.dma_start(outw_d[v0_:v1_, r0:r1, :], woutv)
                    nc.sync.dma_start(outi_d[v0_:v1_, r0:r1, :], ioutv)

    nc.compile()
    return nc


def make_consts(rc=RC):
    eye = np.eye(K0, dtype=bool)
    kmask = eye[:, None, :] | eye[None, :, :]           # (i,j,k): k==i or k==j
    eyeE = np.where(kmask, BIG, -BIG).astype(np.float32).reshape(1, K3)
    neq = (~eye).astype(np.float32).reshape(1, K2)
    iota8 = np.arange(K0, dtype=np.float32).reshape(1, K0)
    iotaI = (np.arange(K2) // K0).astype(np.float32).reshape(1, K2)
    iotaJ = (np.arange(K2) % K0).astype(np.float32).reshape(1, K2)
    iota64ij = np.repeat(np.arange(K2, dtype=np.float32), rc).reshape(1, K2 * rc)
    return {"eyeE": eyeE, "neq": neq, "iota8": iota8, "iotaI": iotaI,
            "iotaJ": iotaJ, "iota64ij": iota64ij}


def make_in_maps(template, projections):
    template = np.ascontiguousarray(np.asarray(template, np.float32))
    projections = np.ascontiguousarray(np.asarray(projections, np.float32))
    consts = make_consts()
    tmplT = np.stack([template[..., 0].reshape(-1), template[..., 1].reshape(-1)])
    px_all = np.ascontiguousarray(projections[..., 0])   # (5000, 8)
    py_all = np.ascontiguousarray(projections[..., 1])
    in_maps = []
    for c in range(N_CORES):
        pxc = px_all[c * VS:(c + 1) * VS]
        pyc = py_all[c * VS:(c + 1) * VS]
        pad = VSP - VS
        pxc = np.concatenate([pxc, np.broadcast_to(pxc[:1], (pad, K0))], 0)
        pyc = np.concatenate([pyc, np.broadcast_to(pyc[:1], (pad, K0))], 0)
        m = {"px": np.ascontiguousarray(pxc), "py": np.ascontiguousarray(pyc),
             "tmpl": tmplT}
        m.update(consts)
        in_maps.append(m)
    return in_maps


_NC_CACHE = {}


def kernel(template, projections, _want_time=False):
    from concourse.bass_utils import run_bass_kernel_spmd
    if "nc" not in _NC_CACHE:
        _NC_CACHE["nc"] = build_nc()
    nc = _NC_CACHE["nc"]
    in_maps = make_in_maps(template, projections)
    res = run_bass_kernel_spmd(nc, in_maps, core_ids=list(range(N_CORES)))
    ws, idxs = [], []
    for c in range(N_CORES):
        out = res.results[c]
        ws.append(out["outw"][:VS].reshape(VS, R, A, 3))
        idxs.append(out["outi"][:VS].reshape(VS, R, A, 3))
    w = np.concatenate(ws, 0).astype(np.float32)
    idx = np.rint(np.concatenate(idxs, 0)).astype(np.int32)
    if _want_time:
        return (w, idx), res
    return w, idx


# revision 16
# speedup vs baseline: 40.3771x; 40.3771x over previous
"""Trainium2 Bass kernel for BarycentricCoordinates (retrieval_knn).

Per (v, r, a) problem: nearest-neighbor ordering of 8 projected points vs a
template vertex, barycentric weights for every candidate (second, third)
vertex pair, Delaunay empty-circumcircle filter, min-score pair selection.

Algorithm in ORIGINAL k0-index space (no argsort/gather on device):
 - closest point c = argmin_k d2[k] via min-reduce + one-hot equality
 - all 64 ordered pairs (i,j) are candidates; i==j, i==c, j==c masked.
   The Delaunay orientation test (det' >= 0 for all k) keeps at most one
   ordering of each unordered pair == the reference's tie-break (validated
   exactly vs reference on CPU). det' is exactly 0 for k in {i,j,c} (the
   difference rows vanish), so no explicit k-mask is needed.
 - reciprocal is clamped to +-BIG so no NaNs arise anywhere; masked selects
   become plain arithmetic (min/max with +-BIG sentinels).
 - fallback (all candidates invalid): weights=0, indices=[c, o1, o1].

Layout: partitions = 128 v's per tile; free layouts are r-major --
(r, k), (r, ij), (r, ij, k) -- so reduces are contiguous and every operand
view fits the 3-free-dim ISA AP limit. Det cross terms U1/U2/U3 depend only
on (i,j,k): hoisted, computed once per v-tile. 8 cores data-parallel over V.
"""

import sys

sys.path.insert(0, "/opt/trn_rl_repo")

import numpy as np

import concourse.bass as bass
import concourse.bacc as bacc
import concourse.mybir as mybir
from concourse.tile import TileContext

F32 = mybir.dt.float32
I32 = mybir.dt.int32
OP = mybir.AluOpType
AF = mybir.ActivationFunctionType
AX = mybir.AxisListType

BIG = 2.0e38
N_CORES = 8
V_TOTAL = 5000
R, A, K0 = 5, 8, 8
RA = R * A
VS = V_TOTAL // N_CORES
P = 128
VSP = 640
RC = 20
K2 = 64
K3 = 512


def build_nc(vsp=VSP, rc=RC, ra=RA):
    nc = bacc.Bacc("TRN2", target_bir_lowering=False)
    n_vt = vsp // P
    n_rch = ra // rc

    px_d = nc.dram_tensor("px", (vsp, K0), F32, kind="ExternalInput")
    py_d = nc.dram_tensor("py", (vsp, K0), F32, kind="ExternalInput")
    tmpl_d = nc.dram_tensor("tmpl", (2, ra), F32, kind="ExternalInput")
    neq_d = nc.dram_tensor("neq", (1, K2), F32, kind="ExternalInput")
    iota8_d = nc.dram_tensor("iota8", (1, K0), F32, kind="ExternalInput")
    iotaI_d = nc.dram_tensor("iotaI", (1, K2), F32, kind="ExternalInput")
    iotaJ_d = nc.dram_tensor("iotaJ", (1, K2), F32, kind="ExternalInput")
    iota64_d = nc.dram_tensor("iota64", (1, K2), F32, kind="ExternalInput")
    outw_d = nc.dram_tensor("outw", (vsp, ra, 3), F32, kind="ExternalOutput")
    outi_d = nc.dram_tensor("outi", (vsp, ra, 3), F32, kind="ExternalOutput")

    with TileContext(nc) as tc:
        VE = nc.vector
        GP = nc.gpsimd
        SC = nc.scalar
        S = rc * K3
        PP = rc * K2
        RK = rc * K0

        with (
            tc.tile_pool(name="const", bufs=1) as cpool,
            tc.tile_pool(name="vt", bufs=2) as vpool,
            tc.tile_pool(name="det", bufs=2) as spool,
            tc.tile_pool(name="pair", bufs=1) as ppool,
            tc.tile_pool(name="pair2", bufs=2) as ppool2,
            tc.tile_pool(name="rk", bufs=1) as rkpool,
            tc.tile_pool(name="small", bufs=2) as opool,
        ):
            TX = cpool.tile([P, ra], F32, tag="TX")
            TY = cpool.tile([P, ra], F32, tag="TY")
            NEQ = cpool.tile([P, K2], F32, tag="NEQ")
            IOTA8 = cpool.tile([P, K0], F32, tag="IOTA8")
            IOTAI = cpool.tile([P, K2], F32, tag="IOTAI")
            IOTAJ = cpool.tile([P, K2], F32, tag="IOTAJ")
            IOTA64 = cpool.tile([P, K2], F32, tag="IOTA64")
            nc.sync.dma_start(TX, tmpl_d[0:1, :].to_broadcast((P, ra)))
            nc.sync.dma_start(TY, tmpl_d[1:2, :].to_broadcast((P, ra)))
            nc.sync.dma_start(NEQ, neq_d[0:1, :].to_broadcast((P, K2)))
            nc.sync.dma_start(IOTA8, iota8_d[0:1, :].to_broadcast((P, K0)))
            nc.sync.dma_start(IOTAI, iotaI_d[0:1, :].to_broadcast((P, K2)))
            nc.sync.dma_start(IOTAJ, iotaJ_d[0:1, :].to_broadcast((P, K2)))
            nc.sync.dma_start(IOTA64, iota64_d[0:1, :].to_broadcast((P, K2)))

            def bcv(ap, shape):
                return ap.to_broadcast(shape)

            for vt in range(n_vt):
                v0_, v1_ = vt * P, (vt + 1) * P
                px = vpool.tile([P, K0], F32, tag="px")
                py = vpool.tile([P, K0], F32, tag="py")
                nc.sync.dma_start(px, px_d[v0_:v1_, :])
                nc.sync.dma_start(py, py_d[v0_:v1_, :])

                s_ = vpool.tile([P, K0], F32, tag="s")
                t8 = vpool.tile([P, K0], F32, tag="t8")
                VE.tensor_tensor(out=s_, in0=px, in1=px, op=OP.mult)
                GP.tensor_tensor(out=t8, in0=py, in1=py, op=OP.mult)
                VE.tensor_tensor(out=s_, in0=s_, in1=t8, op=OP.add)

                PXYS = vpool.tile([P, 32], F32, tag="PXYS")
                SC.copy(out=PXYS[:, 0:8], in_=px)
                SC.copy(out=PXYS[:, 8:16], in_=py)
                SC.copy(out=PXYS[:, 16:24], in_=s_)
                SC.copy(out=PXYS[:, 24:32], in_=IOTA8)

                # b-tensors (i,k), then hoisted det cross terms U (i,j,k):
                # u1 = by_i*bs_j - bs_i*by_j, u2 = bx_i*bs_j - bs_i*bx_j,
                # u3 = bx_i*by_j - by_i*bx_j
                bx = vpool.tile([P, K2], F32, tag="bx")
                by = vpool.tile([P, K2], F32, tag="by")
                bs = vpool.tile([P, K2], F32, tag="bs")
                bxv = bx.rearrange("p (i k) -> p i k", k=K0)
                byv = by.rearrange("p (i k) -> p i k", k=K0)
                bsv = bs.rearrange("p (i k) -> p i k", k=K0)
                VE.tensor_tensor(out=bxv, in0=bcv(px.unsqueeze(2), (P, K0, K0)),
                                 in1=bcv(px.unsqueeze(1), (P, K0, K0)), op=OP.subtract)
                GP.tensor_tensor(out=byv, in0=bcv(py.unsqueeze(2), (P, K0, K0)),
                                 in1=bcv(py.unsqueeze(1), (P, K0, K0)), op=OP.subtract)
                VE.tensor_tensor(out=bsv, in0=bcv(s_.unsqueeze(2), (P, K0, K0)),
                                 in1=bcv(s_.unsqueeze(1), (P, K0, K0)), op=OP.subtract)

                def Bi(t):
                    return bcv(t.rearrange("p (i k) -> p i k", k=K0).unsqueeze(2),
                               (P, K0, K0, K0))

                def Bj(t):
                    return bcv(t.rearrange("p (j k) -> p j k", k=K0).unsqueeze(1),
                               (P, K0, K0, K0))

                U1 = vpool.tile([P, K3], F32, tag="U1")
                U2 = vpool.tile([P, K3], F32, tag="U2")
                U3 = vpool.tile([P, K3], F32, tag="U3")
                uA = vpool.tile([P, K3], F32, tag="uA")
                U1v = U1.rearrange("p (i j k) -> p i j k", j=K0, k=K0)
                U2v = U2.rearrange("p (i j k) -> p i j k", j=K0, k=K0)
                U3v = U3.rearrange("p (i j k) -> p i j k", j=K0, k=K0)
                uAv = uA.rearrange("p (i j k) -> p i j k", j=K0, k=K0)
                VE.tensor_tensor(out=U1v, in0=Bi(by), in1=Bj(bs), op=OP.mult)
                GP.tensor_tensor(out=uAv, in0=Bi(bs), in1=Bj(by), op=OP.mult)
                VE.tensor_tensor(out=U1, in0=U1, in1=uA, op=OP.subtract)
                GP.tensor_tensor(out=U2v, in0=Bi(bx), in1=Bj(bs), op=OP.mult)
                VE.tensor_tensor(out=uAv, in0=Bi(bs), in1=Bj(bx), op=OP.mult)
                GP.tensor_tensor(out=U2, in0=U2, in1=uA, op=OP.subtract)
                VE.tensor_tensor(out=U3v, in0=Bi(bx), in1=Bj(by), op=OP.mult)
                GP.tensor_tensor(out=uAv, in0=Bi(by), in1=Bj(bx), op=OP.mult)
                VE.tensor_tensor(out=U3, in0=U3, in1=uA, op=OP.subtract)

                for rchunk in range(n_rch):
                    r0 = rchunk * rc
                    r1 = r0 + rc
                    # ---------- stage A: distances, closest; (r, k) ----------
                    d2 = rkpool.tile([P, RK], F32, tag="d2")
                    tdx = rkpool.tile([P, RK], F32, tag="tdx")
                    tdy = rkpool.tile([P, RK], F32, tag="tdy")
                    d2v = d2.rearrange("p (r k) -> p r k", k=K0)
                    tdxv = tdx.rearrange("p (r k) -> p r k", k=K0)
                    tdyv = tdy.rearrange("p (r k) -> p r k", k=K0)
                    px_rk = bcv(px.unsqueeze(1), (P, rc, K0))
                    py_rk = bcv(py.unsqueeze(1), (P, rc, K0))
                    s_rk = bcv(s_.unsqueeze(1), (P, rc, K0))
                    tx_rk = bcv(TX[:, r0:r1].unsqueeze(2), (P, rc, K0))
                    ty_rk = bcv(TY[:, r0:r1].unsqueeze(2), (P, rc, K0))
                    VE.tensor_tensor(out=tdxv, in0=px_rk, in1=tx_rk, op=OP.subtract)
                    GP.tensor_tensor(out=tdyv, in0=py_rk, in1=ty_rk, op=OP.subtract)
                    VE.tensor_tensor(out=tdx, in0=tdx, in1=tdx, op=OP.mult)
                    GP.tensor_tensor(out=tdy, in0=tdy, in1=tdy, op=OP.mult)
                    VE.tensor_tensor(out=d2, in0=tdx, in1=tdy, op=OP.add)

                    dmin = opool.tile([P, rc], F32, tag="dmin")
                    VE.tensor_reduce(out=dmin, in_=d2v, axis=AX.X, op=OP.min)
                    dmin_rk = bcv(dmin.unsqueeze(2), (P, rc, K0))
                    m0 = rkpool.tile([P, RK], F32, tag="m0")
                    m0v = m0.rearrange("p (r k) -> p r k", k=K0)
                    VE.tensor_tensor(out=m0v, in0=d2v, in1=dmin_rk, op=OP.is_equal)
                    dgt = rkpool.tile([P, RK], F32, tag="dgt")
                    dgtv = dgt.rearrange("p (r k) -> p r k", k=K0)
                    VE.tensor_tensor(out=dgtv, in0=d2v, in1=dmin_rk, op=OP.is_gt)

                    tA4 = rkpool.tile([P, RK * 4], F32, tag="tA4")
                    tA4v = tA4.rearrange("p (r g k) -> p r g k", g=4, k=K0)
                    tA4r = rkpool.tile([P, rc * 4], F32, tag="tA4r")
                    tA4rv = tA4r.rearrange("p (r g) -> p r g", g=4)
                    m0_rgk = bcv(m0v.unsqueeze(2), (P, rc, 4, K0))
                    pxys_rgk = bcv(PXYS.rearrange("p (g k) -> p g k", k=K0)
                                   .unsqueeze(1), (P, rc, 4, K0))
                    GP.tensor_tensor(out=tA4v, in0=m0_rgk, in1=pxys_rgk, op=OP.mult)
                    VE.tensor_reduce(out=tA4rv, in_=tA4v, axis=AX.X, op=OP.add)
                    cx = tA4rv[:, :, 0].squeeze(2)
                    cy = tA4rv[:, :, 1].squeeze(2)
                    sc_ = tA4rv[:, :, 2].squeeze(2)
                    c_f = tA4rv[:, :, 3].squeeze(2)
                    o1_f = opool.tile([P, rc], F32, tag="o1_f")
                    tA = rkpool.tile([P, RK], F32, tag="tA")
                    tAv = tA.rearrange("p (r k) -> p r k", k=K0)
                    i8_rk = bcv(IOTA8.unsqueeze(1), (P, rc, K0))
                    d2b = rkpool.tile([P, RK], F32, tag="d2b")
                    VE.scalar_tensor_tensor(out=d2b, in0=m0, scalar=BIG, in1=d2,
                                            op0=OP.mult, op1=OP.add)
                    dmin2 = opool.tile([P, rc], F32, tag="dmin2")
                    d2bv = d2b.rearrange("p (r k) -> p r k", k=K0)
                    VE.tensor_reduce(out=dmin2, in_=d2bv, axis=AX.X, op=OP.min)
                    dmin2_rk = bcv(dmin2.unsqueeze(2), (P, rc, K0))
                    VE.tensor_tensor(out=tAv, in0=d2bv, in1=dmin2_rk, op=OP.is_equal)
                    GP.tensor_tensor(out=tAv, in0=tAv, in1=i8_rk, op=OP.mult)
                    VE.tensor_reduce(out=o1_f, in_=tAv, axis=AX.X, op=OP.add)

                    # ---------- stage B: v0, dots; (r, k) ----------
                    v0x = rkpool.tile([P, RK], F32, tag="v0x")
                    v0y = rkpool.tile([P, RK], F32, tag="v0y")
                    ass = rkpool.tile([P, RK], F32, tag="ass")
                    d00 = rkpool.tile([P, RK], F32, tag="d00")
                    d02 = rkpool.tile([P, RK], F32, tag="d02")
                    v0xv = v0x.rearrange("p (r k) -> p r k", k=K0)
                    v0yv = v0y.rearrange("p (r k) -> p r k", k=K0)
                    assv = ass.rearrange("p (r k) -> p r k", k=K0)
                    d02v = d02.rearrange("p (r k) -> p r k", k=K0)
                    cx_rk = bcv(cx.unsqueeze(2), (P, rc, K0))
                    cy_rk = bcv(cy.unsqueeze(2), (P, rc, K0))
                    sc_rk = bcv(sc_.unsqueeze(2), (P, rc, K0))
                    VE.tensor_tensor(out=v0xv, in0=px_rk, in1=cx_rk, op=OP.subtract)
                    GP.tensor_tensor(out=v0yv, in0=py_rk, in1=cy_rk, op=OP.subtract)
                    GP.tensor_tensor(out=assv, in0=s_rk, in1=sc_rk, op=OP.subtract)
                    v2x = opool.tile([P, rc], F32, tag="v2x")
                    v2y = opool.tile([P, rc], F32, tag="v2y")
                    VE.tensor_tensor(out=v2x, in0=TX[:, r0:r1], in1=cx, op=OP.subtract)
                    VE.tensor_tensor(out=v2y, in0=TY[:, r0:r1], in1=cy, op=OP.subtract)
                    tB = rkpool.tile([P, RK], F32, tag="tB")
                    tBv = tB.rearrange("p (r k) -> p r k", k=K0)
                    VE.tensor_tensor(out=d00, in0=v0x, in1=v0x, op=OP.mult)
                    GP.tensor_tensor(out=tB, in0=v0y, in1=v0y, op=OP.mult)
                    VE.tensor_tensor(out=d00, in0=d00, in1=tB, op=OP.add)
                    v2x_rk = bcv(v2x.unsqueeze(2), (P, rc, K0))
                    v2y_rk = bcv(v2y.unsqueeze(2), (P, rc, K0))
                    VE.tensor_tensor(out=d02v, in0=v0xv, in1=v2x_rk, op=OP.mult)
                    GP.tensor_tensor(out=tBv, in0=v0yv, in1=v2y_rk, op=OP.mult)
                    VE.tensor_tensor(out=d02, in0=d02, in1=tB, op=OP.add)

                    # ---------- stage C: delaunay; (r, ij, k) ----------
                    # det' = v0x(r,k)*U1(ij,k) - v0y(r,k)*U2(ij,k) + ass(r,k)*U3(ij,k)
                    # det' == 0 automatically for k in {i, j, c}: passes >= 0.
                    def Uv(t):   # (ij*k) -> (r, ij, k), broadcast r
                        return bcv(t.rearrange("p (ij k) -> p ij k", k=K0).unsqueeze(1),
                                   (P, rc, K2, K0))

                    def Kv(t2):  # (r,k) -> (r, ij, k), broadcast ij
                        return bcv(t2.rearrange("p (r k) -> p r k", k=K0).unsqueeze(2),
                                   (P, rc, K2, K0))

                    mindet = ppool.tile([P, PP], F32, tag="mindet")
                    mindet_rij = mindet.rearrange("p (r ij) -> p r ij", ij=K2)
                    HIJ = K2 // 2
                    for h in range(2):
                        ij0 = h * HIJ

                        def Uvh(t):
                            return bcv(t.rearrange("p (ij k) -> p ij k", k=K0)
                                       [:, ij0:ij0 + HIJ, :].unsqueeze(1),
                                       (P, rc, HIJ, K0))

                        def Kvh(t2):
                            return bcv(t2.rearrange("p (r k) -> p r k", k=K0)
                                       .unsqueeze(2), (P, rc, HIJ, K0))

                        sA = spool.tile([P, S // 2], F32, tag="sA")
                        sB = spool.tile([P, S // 2], F32, tag="sB")
                        sAv = sA.rearrange("p (r ij k) -> p r ij k", ij=HIJ, k=K0)
                        sBv = sB.rearrange("p (r ij k) -> p r ij k", ij=HIJ, k=K0)
                        VE.tensor_tensor(out=sAv, in0=Kvh(v0x), in1=Uvh(U1), op=OP.mult)
                        GP.tensor_tensor(out=sBv, in0=Kvh(v0y), in1=Uvh(U2), op=OP.mult)
                        VE.tensor_tensor(out=sA, in0=sA, in1=sB, op=OP.subtract)
                        GP.tensor_tensor(out=sBv, in0=Kvh(ass), in1=Uvh(U3), op=OP.mult)
                        VE.tensor_tensor(out=sA, in0=sA, in1=sB, op=OP.add)
                        VE.tensor_reduce(out=mindet_rij[:, :, ij0:ij0 + HIJ],
                                         in_=sAv, axis=AX.X, op=OP.min)

                    # ---------- stage D: pair weights; (r, ij)=(r, i, j) ----------
                    # (r,k) tiles as (r, i, j): i from k slot / j broadcast, or
                    # j from k slot / i broadcast.
                    def XI(t2):
                        return bcv(t2.rearrange("p (r k) -> p r k", k=K0).unsqueeze(3),
                                   (P, rc, K0, K0))

                    def XJ(t2):
                        return bcv(t2.rearrange("p (r k) -> p r k", k=K0).unsqueeze(2),
                                   (P, rc, K0, K0))

                    dot01 = ppool2.tile([P, PP], F32, tag="dot01")
                    pA = ppool2.tile([P, PP], F32, tag="pA")
                    pB = ppool2.tile([P, PP], F32, tag="pB")
                    w1t = ppool.tile([P, PP], F32, tag="w1t")
                    w2t = ppool.tile([P, PP], F32, tag="w2t")
                    w0t = ppool.tile([P, PP], F32, tag="w0t")
                    inv = ppool.tile([P, PP], F32, tag="inv")
                    dot01v = dot01.rearrange("p (r i j) -> p r i j", i=K0, j=K0)
                    pAv = pA.rearrange("p (r i j) -> p r i j", i=K0, j=K0)
                    pBv = pB.rearrange("p (r i j) -> p r i j", i=K0, j=K0)

                    VE.tensor_tensor(out=dot01v, in0=XI(v0x), in1=XJ(v0x), op=OP.mult)
                    GP.tensor_tensor(out=pAv, in0=XI(v0y), in1=XJ(v0y), op=OP.mult)
                    VE.tensor_tensor(out=dot01, in0=dot01, in1=pA, op=OP.add)
                    GP.tensor_tensor(out=pAv, in0=XI(d00), in1=XJ(d00), op=OP.mult)
                    VE.tensor_tensor(out=pB, in0=dot01, in1=dot01, op=OP.mult)
                    VE.tensor_tensor(out=pA, in0=pA, in1=pB, op=OP.subtract)  # denom
                    VE.reciprocal(out=inv, in_=pA)
                    # newton refine, then clamp to +-BIG (kills inf -> no NaNs later)
                    GP.tensor_tensor(out=pB, in0=pA, in1=inv, op=OP.mult)
                    SC.activation(out=pB, in_=pB, func=AF.Copy, bias=2.0, scale=-1.0)
                    VE.tensor_tensor(out=inv, in0=inv, in1=pB, op=OP.mult)
                    VE.tensor_scalar(out=inv, in0=inv, scalar1=BIG, scalar2=-BIG,
                                     op0=OP.min, op1=OP.max)

                    VE.tensor_tensor(out=pAv, in0=XJ(d00), in1=XI(d02), op=OP.mult)
                    GP.tensor_tensor(out=pBv, in0=dot01v, in1=XJ(d02), op=OP.mult)
                    VE.tensor_tensor(out=w2t, in0=pA, in1=pB, op=OP.subtract)
                    VE.tensor_tensor(out=w2t, in0=w2t, in1=inv, op=OP.mult)
                    GP.tensor_tensor(out=pAv, in0=XI(d00), in1=XJ(d02), op=OP.mult)
                    VE.tensor_tensor(out=pBv, in0=dot01v, in1=XI(d02), op=OP.mult)
                    VE.tensor_tensor(out=w1t, in0=pA, in1=pB, op=OP.subtract)
                    VE.tensor_tensor(out=w1t, in0=w1t, in1=inv, op=OP.mult)
                    VE.tensor_tensor(out=pA, in0=w2t, in1=w1t, op=OP.add)
                    SC.activation(out=w0t, in_=pA, func=AF.Copy, bias=1.0, scale=-1.0)

                    wm = ppool.tile([P, PP], F32, tag="wm")
                    VE.tensor_tensor(out=wm, in0=w1t, in1=w2t, op=OP.min)
                    VE.tensor_tensor(out=wm, in0=wm, in1=w0t, op=OP.min)
                    sq = ppool.tile([P, PP], F32, tag="sq")
                    sr = ppool.tile([P, PP], F32, tag="sr")
                    SC.activation(out=sr, in_=w0t, func=AF.Square)
                    SC.activation(out=sq, in_=w1t, func=AF.Square)
                    VE.tensor_tensor(out=sr, in0=sr, in1=sq, op=OP.max)
                    SC.activation(out=sq, in_=w2t, func=AF.Square)
                    VE.tensor_tensor(out=sr, in0=sr, in1=sq, op=OP.max)

                    # valid = (wm>0)*(mindet>=0)*dgt_i*dgt_j*neq  (fused pairs)
                    valid = ppool.tile([P, PP], F32, tag="valid")
                    validv = valid.rearrange("p (r i j) -> p r i j", i=K0, j=K0)
                    VE.scalar_tensor_tensor(out=validv, in0=wm.rearrange(
                        "p (r i j) -> p r i j", i=K0, j=K0), scalar=0.0,
                        in1=XI(dgt), op0=OP.is_gt, op1=OP.mult)
                    GP.tensor_tensor(out=pBv, in0=XJ(dgt), in1=bcv(
                        NEQ.rearrange("p (i j) -> p i j", j=K0).unsqueeze(1),
                        (P, rc, K0, K0)), op=OP.mult)
                    VE.scalar_tensor_tensor(out=pA, in0=mindet, scalar=0.0,
                                            in1=pB, op0=OP.is_ge, op1=OP.mult)
                    VE.tensor_tensor(out=valid, in0=valid, in1=pA, op=OP.mult)

                    # score = max(sr, BIG*(1-valid)); arithmetic, NaN-free
                    score = ppool.tile([P, PP], F32, tag="score")
                    VE.tensor_scalar(out=score, in0=valid, scalar1=-BIG, scalar2=BIG,
                                     op0=OP.mult, op1=OP.add)
                    VE.tensor_tensor(out=score, in0=score, in1=sr, op=OP.max)
                    scorev = score.rearrange("p (r q) -> p r q", q=K2)
                    smin = opool.tile([P, rc], F32, tag="smin")
                    VE.tensor_reduce(out=smin, in_=scorev, axis=AX.X, op=OP.min)
                    smin_q = bcv(smin.unsqueeze(2), (P, rc, K2))
                    eqm = ppool.tile([P, PP], F32, tag="eqm")
                    eqmv = eqm.rearrange("p (r q) -> p r q", q=K2)
                    VE.tensor_tensor(out=eqmv, in0=scorev, in1=smin_q, op=OP.is_equal)
                    # pidt = iota64 where eqm else BIG  ->  first-index argmin
                    pidt = ppool.tile([P, PP], F32, tag="score")
                    pidtv = pidt.rearrange("p (r q) -> p r q", q=K2)
                    VE.tensor_scalar(out=pidt, in0=eqm, scalar1=-BIG, scalar2=BIG,
                                     op0=OP.mult, op1=OP.add)
                    i64_q = bcv(IOTA64.unsqueeze(1), (P, rc, K2))
                    VE.tensor_tensor(out=pidtv, in0=pidtv, in1=i64_q, op=OP.max)
                    pidx = opool.tile([P, rc], F32, tag="pidx")
                    VE.tensor_reduce(out=pidx, in_=pidtv, axis=AX.X, op=OP.min)
                    pidx_q = bcv(pidx.unsqueeze(2), (P, rc, K2))
                    oh = ppool.tile([P, PP], F32, tag="eqm")
                    ohv_ = oh.rearrange("p (r q) -> p r q", q=K2)
                    VE.tensor_tensor(out=ohv_, in0=i64_q, in1=pidx_q, op=OP.is_equal)

                    # gathers: weights (NaN-free now), pair indices
                    g0 = ppool2.tile([P, PP], F32, tag="dot01")
                    g0v = g0.rearrange("p (r q) -> p r q", q=K2)
                    w0sel = opool.tile([P, rc], F32, tag="w0sel")
                    w1sel = opool.tile([P, rc], F32, tag="w1sel")
                    w2sel = opool.tile([P, rc], F32, tag="w2sel")
                    i_f = opool.tile([P, rc], F32, tag="i_f")
                    j_f = opool.tile([P, rc], F32, tag="j_f")
                    GP.tensor_tensor(out=g0, in0=oh, in1=w0t, op=OP.mult)
                    VE.tensor_reduce(out=w0sel, in_=g0v, axis=AX.X, op=OP.add)
                    GP.tensor_tensor(out=g0, in0=oh, in1=w2t, op=OP.mult)
                    VE.tensor_reduce(out=w2sel, in_=g0v, axis=AX.X, op=OP.add)
                    GP.tensor_tensor(out=g0, in0=oh, in1=w1t, op=OP.mult)
                    VE.tensor_reduce(out=w1sel, in_=g0v, axis=AX.X, op=OP.add)
                    i64I = bcv(IOTAI.unsqueeze(1), (P, rc, K2))
                    i64J = bcv(IOTAJ.unsqueeze(1), (P, rc, K2))
                    GP.tensor_tensor(out=g0v, in0=ohv_, in1=i64I, op=OP.mult)
                    VE.tensor_reduce(out=i_f, in_=g0v, axis=AX.X, op=OP.add)
                    GP.tensor_tensor(out=g0v, in0=ohv_, in1=i64J, op=OP.mult)
                    VE.tensor_reduce(out=j_f, in_=g0v, axis=AX.X, op=OP.add)

                    # fallback: all candidates invalid -> w=0, idx=[c, o1, o1]
                    fb = opool.tile([P, rc], F32, tag="fb")
                    nfb = opool.tile([P, rc], F32, tag="nfb")
                    VE.tensor_scalar(out=fb, in0=smin, scalar1=1.0e38, scalar2=None,
                                     op0=OP.is_ge)
                    VE.tensor_scalar(out=nfb, in0=fb, scalar1=-1.0, scalar2=1.0,
                                     op0=OP.mult, op1=OP.add)
                    VE.tensor_tensor(out=w0sel, in0=w0sel, in1=nfb, op=OP.mult)
                    VE.tensor_tensor(out=w1sel, in0=w1sel, in1=nfb, op=OP.mult)
                    VE.tensor_tensor(out=w2sel, in0=w2sel, in1=nfb, op=OP.mult)
                    VE.copy_predicated(out=i_f, mask=fb.bitcast(I32), data=o1_f)
                    VE.copy_predicated(out=j_f, mask=fb.bitcast(I32), data=o1_f)

                    # ---------- outputs ----------
                    wout = opool.tile([P, rc * 3], F32, tag="wout")
                    iout = opool.tile([P, rc * 3], F32, tag="iout")
                    woutv = wout.rearrange("p (r c) -> p r c", c=3)
                    ioutv = iout.rearrange("p (r c) -> p r c", c=3)
                    SC.copy(out=woutv[:, :, 0], in_=w0sel)
                    SC.copy(out=woutv[:, :, 1], in_=w2sel)
                    SC.copy(out=woutv[:, :, 2], in_=w1sel)
                    SC.copy(out=ioutv[:, :, 0], in_=c_f)
                    SC.copy(out=ioutv[:, :, 1], in_=i_f)
                    SC.copy(out=ioutv[:, :, 2], in_=j_f)
                    nc.sync.dma_start(outw_d[v0_:v1_, r0:r1, :], woutv)
                    nc.sync.dma_start(outi_d[v0_:v1_, r0:r1, :], ioutv)

    nc.compile()
    return nc


def make_consts(rc=RC):
    eye = np.eye(K0, dtype=bool)
    neq = (~eye).astype(np.float32).reshape(1, K2)
    iota8 = np.arange(K0, dtype=np.float32).reshape(1, K0)
    iotaI = (np.arange(K2) // K0).astype(np.float32).reshape(1, K2)
    iotaJ = (np.arange(K2) % K0).astype(np.float32).reshape(1, K2)
    iota64 = np.arange(K2, dtype=np.float32).reshape(1, K2)
    return {"neq": neq, "iota8": iota8, "iotaI": iotaI, "iotaJ": iotaJ,
            "iota64": iota64}


def make_in_maps(template, projections):
    template = np.ascontiguousarray(np.asarray(template, np.float32))
    projections = np.ascontiguousarray(np.asarray(projections, np.float32))
    consts = make_consts()
    tmplT = np.stack([template[..., 0].reshape(-1), template[..., 1].reshape(-1)])
    px_all = np.ascontiguousarray(projections[..., 0])
    py_all = np.ascontiguousarray(projections[..., 1])
    in_maps = []
    for c in range(N_CORES):
        pxc = px_all[c * VS:(c + 1) * VS]
        pyc = py_all[c * VS:(c + 1) * VS]
        pad = VSP - VS
        pxc = np.concatenate([pxc, np.broadcast_to(pxc[:1], (pad, K0))], 0)
        pyc = np.concatenate([pyc, np.broadcast_to(pyc[:1], (pad, K0))], 0)
        m = {"px": np.ascontiguousarray(pxc), "py": np.ascontiguousarray(pyc),
             "tmpl": tmplT}
        m.update(consts)
        in_maps.append(m)
    return in_maps


_NC_CACHE = {}


def kernel(template, projections, _want_time=False):
    from concourse.bass_utils import run_bass_kernel_spmd
    if "nc" not in _NC_CACHE:
        _NC_CACHE["nc"] = build_nc()
    nc = _NC_CACHE["nc"]
    in_maps = make_in_maps(template, projections)
    res = run_bass_kernel_spmd(nc, in_maps, core_ids=list(range(N_CORES)))
    ws, idxs = [], []
    for c in range(N_CORES):
        out = res.results[c]
        ws.append(out["outw"][:VS].reshape(VS, R, A, 3))
        idxs.append(out["outi"][:VS].reshape(VS, R, A, 3))
    w = np.concatenate(ws, 0).astype(np.float32)
    idx = np.rint(np.concatenate(idxs, 0)).astype(np.int32)
    if _want_time:
        return (w, idx), res
    return w, idx


# revision 18
# speedup vs baseline: 49.9579x; 1.2373x over previous
"""Trainium2 Bass kernel for BarycentricCoordinates (retrieval_knn).

Per (v, r, a) problem: nearest-neighbor ordering of 8 projected points vs a
template vertex, barycentric weights for every candidate (second, third)
vertex pair, Delaunay empty-circumcircle filter, min-score pair selection.

Algorithm in ORIGINAL k0-index space (no argsort/gather on device):
 - closest point c = argmin_k d2[k] via min-reduce + one-hot equality
 - all 64 ordered pairs (i,j) are candidates; i==j, i==c, j==c masked.
   The Delaunay orientation test (det' >= 0 for all k) keeps at most one
   ordering of each unordered pair == the reference's tie-break (validated
   exactly vs reference on CPU). det' is exactly 0 for k in {i,j,c} (the
   difference rows vanish), so no explicit k-mask is needed.
 - reciprocal is clamped to +-BIG so no NaNs arise anywhere; masked selects
   become plain arithmetic (min/max with +-BIG sentinels).
 - fallback (all candidates invalid): weights=0, indices=[c, o1, o1].

Layout: partitions = 128 v's per tile; free layouts are r-major --
(r, k), (r, ij), (r, ij, k) -- so reduces are contiguous and every operand
view fits the 3-free-dim ISA AP limit. Det cross terms U1/U2/U3 depend only
on (i,j,k): hoisted, computed once per v-tile. 8 cores data-parallel over V.
"""

import sys

sys.path.insert(0, "/opt/trn_rl_repo")

import numpy as np

import concourse.bass as bass
import concourse.bacc as bacc
import concourse.mybir as mybir
from concourse.tile import TileContext

F32 = mybir.dt.float32
I32 = mybir.dt.int32
OP = mybir.AluOpType
AF = mybir.ActivationFunctionType
AX = mybir.AxisListType

BIG = 2.0e38
N_CORES = 8
V_TOTAL = 5000
R, A, K0 = 5, 8, 8
RA = R * A
VS = V_TOTAL // N_CORES
P = 128
VSP = 640
RC = 20
K2 = 64
K3 = 512


def build_nc(vsp=VSP, rc=RC, ra=RA):
    nc = bacc.Bacc("TRN2", target_bir_lowering=False)
    n_vt = vsp // P
    n_rch = ra // rc

    px_d = nc.dram_tensor("px", (vsp, K0), F32, kind="ExternalInput")
    py_d = nc.dram_tensor("py", (vsp, K0), F32, kind="ExternalInput")
    tmpl_d = nc.dram_tensor("tmpl", (2, ra), F32, kind="ExternalInput")
    neq_d = nc.dram_tensor("neq", (1, K2), F32, kind="ExternalInput")
    iota8_d = nc.dram_tensor("iota8", (1, K0), F32, kind="ExternalInput")
    iotaI_d = nc.dram_tensor("iotaI", (1, K2), F32, kind="ExternalInput")
    iotaJ_d = nc.dram_tensor("iotaJ", (1, K2), F32, kind="ExternalInput")
    iota64_d = nc.dram_tensor("iota64", (1, K2), F32, kind="ExternalInput")
    outw_d = nc.dram_tensor("outw", (vsp, ra, 3), F32, kind="ExternalOutput")
    outi_d = nc.dram_tensor("outi", (vsp, ra, 3), F32, kind="ExternalOutput")

    with TileContext(nc) as tc:
        VE = nc.vector
        GP = nc.gpsimd
        SC = nc.scalar
        S = rc * K3
        PP = rc * K2
        RK = rc * K0

        with (
            tc.tile_pool(name="const", bufs=1) as cpool,
            tc.tile_pool(name="vt", bufs=2) as vpool,
            tc.tile_pool(name="det", bufs=2) as spool,
            tc.tile_pool(name="pair", bufs=1) as ppool,
            tc.tile_pool(name="pair2", bufs=2) as ppool2,
            tc.tile_pool(name="rk", bufs=1) as rkpool,
            tc.tile_pool(name="small", bufs=2) as opool,
        ):
            TX = cpool.tile([P, ra], F32, tag="TX")
            TY = cpool.tile([P, ra], F32, tag="TY")
            NEQ = cpool.tile([P, K2], F32, tag="NEQ")
            IOTA8 = cpool.tile([P, K0], F32, tag="IOTA8")
            IOTAI = cpool.tile([P, K2], F32, tag="IOTAI")
            IOTAJ = cpool.tile([P, K2], F32, tag="IOTAJ")
            IOTA64 = cpool.tile([P, K2], F32, tag="IOTA64")
            nc.sync.dma_start(TX, tmpl_d[0:1, :].to_broadcast((P, ra)))
            nc.sync.dma_start(TY, tmpl_d[1:2, :].to_broadcast((P, ra)))
            nc.sync.dma_start(NEQ, neq_d[0:1, :].to_broadcast((P, K2)))
            nc.sync.dma_start(IOTA8, iota8_d[0:1, :].to_broadcast((P, K0)))
            nc.sync.dma_start(IOTAI, iotaI_d[0:1, :].to_broadcast((P, K2)))
            nc.sync.dma_start(IOTAJ, iotaJ_d[0:1, :].to_broadcast((P, K2)))
            nc.sync.dma_start(IOTA64, iota64_d[0:1, :].to_broadcast((P, K2)))

            def bcv(ap, shape):
                return ap.to_broadcast(shape)

            for vt in range(n_vt):
                v0_, v1_ = vt * P, (vt + 1) * P
                px = vpool.tile([P, K0], F32, tag="px")
                py = vpool.tile([P, K0], F32, tag="py")
                nc.sync.dma_start(px, px_d[v0_:v1_, :])
                nc.sync.dma_start(py, py_d[v0_:v1_, :])

                s_ = vpool.tile([P, K0], F32, tag="s")
                t8 = vpool.tile([P, K0], F32, tag="t8")
                VE.tensor_tensor(out=s_, in0=px, in1=px, op=OP.mult)
                GP.tensor_tensor(out=t8, in0=py, in1=py, op=OP.mult)
                VE.tensor_tensor(out=s_, in0=s_, in1=t8, op=OP.add)

                PXYS = vpool.tile([P, 32], F32, tag="PXYS")
                SC.copy(out=PXYS[:, 0:8], in_=px)
                SC.copy(out=PXYS[:, 8:16], in_=py)
                SC.copy(out=PXYS[:, 16:24], in_=s_)
                SC.copy(out=PXYS[:, 24:32], in_=IOTA8)

                # b-tensors (i,k), then hoisted det cross terms U (i,j,k):
                # u1 = by_i*bs_j - bs_i*by_j, u2 = bx_i*bs_j - bs_i*bx_j,
                # u3 = bx_i*by_j - by_i*bx_j
                bx = vpool.tile([P, K2], F32, tag="bx")
                by = vpool.tile([P, K2], F32, tag="by")
                bs = vpool.tile([P, K2], F32, tag="bs")
                bxv = bx.rearrange("p (i k) -> p i k", k=K0)
                byv = by.rearrange("p (i k) -> p i k", k=K0)
                bsv = bs.rearrange("p (i k) -> p i k", k=K0)
                VE.tensor_tensor(out=bxv, in0=bcv(px.unsqueeze(2), (P, K0, K0)),
                                 in1=bcv(px.unsqueeze(1), (P, K0, K0)), op=OP.subtract)
                GP.tensor_tensor(out=byv, in0=bcv(py.unsqueeze(2), (P, K0, K0)),
                                 in1=bcv(py.unsqueeze(1), (P, K0, K0)), op=OP.subtract)
                VE.tensor_tensor(out=bsv, in0=bcv(s_.unsqueeze(2), (P, K0, K0)),
                                 in1=bcv(s_.unsqueeze(1), (P, K0, K0)), op=OP.subtract)

                def Bi(t):
                    return bcv(t.rearrange("p (i k) -> p i k", k=K0).unsqueeze(2),
                               (P, K0, K0, K0))

                def Bj(t):
                    return bcv(t.rearrange("p (j k) -> p j k", k=K0).unsqueeze(1),
                               (P, K0, K0, K0))

                U1 = vpool.tile([P, K3], F32, tag="U1")
                U2 = vpool.tile([P, K3], F32, tag="U2")
                U3 = vpool.tile([P, K3], F32, tag="U3")
                uA = vpool.tile([P, K3], F32, tag="uA")
                U1v = U1.rearrange("p (i j k) -> p i j k", j=K0, k=K0)
                U2v = U2.rearrange("p (i j k) -> p i j k", j=K0, k=K0)
                U3v = U3.rearrange("p (i j k) -> p i j k", j=K0, k=K0)
                uAv = uA.rearrange("p (i j k) -> p i j k", j=K0, k=K0)
                VE.tensor_tensor(out=U1v, in0=Bi(by), in1=Bj(bs), op=OP.mult)
                GP.tensor_tensor(out=uAv, in0=Bi(bs), in1=Bj(by), op=OP.mult)
                VE.tensor_tensor(out=U1, in0=U1, in1=uA, op=OP.subtract)
                GP.tensor_tensor(out=U2v, in0=Bi(bx), in1=Bj(bs), op=OP.mult)
                VE.tensor_tensor(out=uAv, in0=Bi(bs), in1=Bj(bx), op=OP.mult)
                GP.tensor_tensor(out=U2, in0=U2, in1=uA, op=OP.subtract)
                VE.tensor_tensor(out=U3v, in0=Bi(bx), in1=Bj(by), op=OP.mult)
                GP.tensor_tensor(out=uAv, in0=Bi(by), in1=Bj(bx), op=OP.mult)
                VE.tensor_tensor(out=U3, in0=U3, in1=uA, op=OP.subtract)

                for rchunk in range(n_rch):
                    r0 = rchunk * rc
                    r1 = r0 + rc
                    # ---------- stage A: distances, closest; (r, k) ----------
                    d2 = rkpool.tile([P, RK], F32, tag="d2")
                    tdx = rkpool.tile([P, RK], F32, tag="tdx")
                    tdy = rkpool.tile([P, RK], F32, tag="tdy")
                    d2v = d2.rearrange("p (r k) -> p r k", k=K0)
                    tdxv = tdx.rearrange("p (r k) -> p r k", k=K0)
                    tdyv = tdy.rearrange("p (r k) -> p r k", k=K0)
                    px_rk = bcv(px.unsqueeze(1), (P, rc, K0))
                    py_rk = bcv(py.unsqueeze(1), (P, rc, K0))
                    s_rk = bcv(s_.unsqueeze(1), (P, rc, K0))
                    tx_rk = bcv(TX[:, r0:r1].unsqueeze(2), (P, rc, K0))
                    ty_rk = bcv(TY[:, r0:r1].unsqueeze(2), (P, rc, K0))
                    VE.tensor_tensor(out=tdxv, in0=px_rk, in1=tx_rk, op=OP.subtract)
                    GP.tensor_tensor(out=tdyv, in0=py_rk, in1=ty_rk, op=OP.subtract)
                    VE.tensor_tensor(out=tdx, in0=tdx, in1=tdx, op=OP.mult)
                    GP.tensor_tensor(out=tdy, in0=tdy, in1=tdy, op=OP.mult)
                    VE.tensor_tensor(out=d2, in0=tdx, in1=tdy, op=OP.add)

                    dmin = opool.tile([P, rc], F32, tag="dmin")
                    VE.tensor_reduce(out=dmin, in_=d2v, axis=AX.X, op=OP.min)
                    dmin_rk = bcv(dmin.unsqueeze(2), (P, rc, K0))
                    m0 = rkpool.tile([P, RK], F32, tag="m0")
                    m0v = m0.rearrange("p (r k) -> p r k", k=K0)
                    VE.tensor_tensor(out=m0v, in0=d2v, in1=dmin_rk, op=OP.is_equal)
                    dgt = rkpool.tile([P, RK], F32, tag="dgt")
                    dgtv = dgt.rearrange("p (r k) -> p r k", k=K0)
                    VE.tensor_tensor(out=dgtv, in0=d2v, in1=dmin_rk, op=OP.is_gt)

                    tA4 = rkpool.tile([P, RK * 4], F32, tag="tA4")
                    tA4v = tA4.rearrange("p (r g k) -> p r g k", g=4, k=K0)
                    tA4r = rkpool.tile([P, rc * 4], F32, tag="tA4r")
                    tA4rv = tA4r.rearrange("p (r g) -> p r g", g=4)
                    m0_rgk = bcv(m0v.unsqueeze(2), (P, rc, 4, K0))
                    pxys_rgk = bcv(PXYS.rearrange("p (g k) -> p g k", k=K0)
                                   .unsqueeze(1), (P, rc, 4, K0))
                    GP.tensor_tensor(out=tA4v, in0=m0_rgk, in1=pxys_rgk, op=OP.mult)
                    VE.tensor_reduce(out=tA4rv, in_=tA4v, axis=AX.X, op=OP.add)
                    cx = tA4rv[:, :, 0].squeeze(2)
                    cy = tA4rv[:, :, 1].squeeze(2)
                    sc_ = tA4rv[:, :, 2].squeeze(2)
                    c_f = tA4rv[:, :, 3].squeeze(2)
                    o1_f = opool.tile([P, rc], F32, tag="o1_f")
                    tA = rkpool.tile([P, RK], F32, tag="tA")
                    tAv = tA.rearrange("p (r k) -> p r k", k=K0)
                    i8_rk = bcv(IOTA8.unsqueeze(1), (P, rc, K0))
                    d2b = rkpool.tile([P, RK], F32, tag="d2b")
                    VE.scalar_tensor_tensor(out=d2b, in0=m0, scalar=BIG, in1=d2,
                                            op0=OP.mult, op1=OP.add)
                    dmin2 = opool.tile([P, rc], F32, tag="dmin2")
                    d2bv = d2b.rearrange("p (r k) -> p r k", k=K0)
                    VE.tensor_reduce(out=dmin2, in_=d2bv, axis=AX.X, op=OP.min)
                    dmin2_rk = bcv(dmin2.unsqueeze(2), (P, rc, K0))
                    VE.tensor_tensor(out=tAv, in0=d2bv, in1=dmin2_rk, op=OP.is_equal)
                    GP.tensor_tensor(out=tAv, in0=tAv, in1=i8_rk, op=OP.mult)
                    VE.tensor_reduce(out=o1_f, in_=tAv, axis=AX.X, op=OP.add)

                    # ---------- stage B: v0, dots; (r, k) ----------
                    v0x = rkpool.tile([P, RK], F32, tag="v0x")
                    v0y = rkpool.tile([P, RK], F32, tag="v0y")
                    ass = rkpool.tile([P, RK], F32, tag="ass")
                    d00 = rkpool.tile([P, RK], F32, tag="d00")
                    d02 = rkpool.tile([P, RK], F32, tag="d02")
                    v0xv = v0x.rearrange("p (r k) -> p r k", k=K0)
                    v0yv = v0y.rearrange("p (r k) -> p r k", k=K0)
                    assv = ass.rearrange("p (r k) -> p r k", k=K0)
                    d02v = d02.rearrange("p (r k) -> p r k", k=K0)
                    cx_rk = bcv(cx.unsqueeze(2), (P, rc, K0))
                    cy_rk = bcv(cy.unsqueeze(2), (P, rc, K0))
                    sc_rk = bcv(sc_.unsqueeze(2), (P, rc, K0))
                    VE.tensor_tensor(out=v0xv, in0=px_rk, in1=cx_rk, op=OP.subtract)
                    GP.tensor_tensor(out=v0yv, in0=py_rk, in1=cy_rk, op=OP.subtract)
                    GP.tensor_tensor(out=assv, in0=s_rk, in1=sc_rk, op=OP.subtract)
                    v2x = opool.tile([P, rc], F32, tag="v2x")
                    v2y = opool.tile([P, rc], F32, tag="v2y")
                    VE.tensor_tensor(out=v2x, in0=TX[:, r0:r1], in1=cx, op=OP.subtract)
                    VE.tensor_tensor(out=v2y, in0=TY[:, r0:r1], in1=cy, op=OP.subtract)
                    tB = rkpool.tile([P, RK], F32, tag="tB")
                    tBv = tB.rearrange("p (r k) -> p r k", k=K0)
                    VE.tensor_tensor(out=d00, in0=v0x, in1=v0x, op=OP.mult)
                    GP.tensor_tensor(out=tB, in0=v0y, in1=v0y, op=OP.mult)
                    VE.tensor_tensor(out=d00, in0=d00, in1=tB, op=OP.add)
                    v2x_rk = bcv(v2x.unsqueeze(2), (P, rc, K0))
                    v2y_rk = bcv(v2y.unsqueeze(2), (P, rc, K0))
                    VE.tensor_tensor(out=d02v, in0=v0xv, in1=v2x_rk, op=OP.mult)
                    GP.tensor_tensor(out=tBv, in0=v0yv, in1=v2y_rk, op=OP.mult)
                    VE.tensor_tensor(out=d02, in0=d02, in1=tB, op=OP.add)

                    # ---------- stage C: delaunay; (r, ij, k) ----------
                    # det' = v0x(r,k)*U1(ij,k) - v0y(r,k)*U2(ij,k) + ass(r,k)*U3(ij,k)
                    # det' == 0 automatically for k in {i, j, c}: passes >= 0.
                    def Uv(t):   # (ij*k) -> (r, ij, k), broadcast r
                        return bcv(t.rearrange("p (ij k) -> p ij k", k=K0).unsqueeze(1),
                                   (P, rc, K2, K0))

                    def Kv(t2):  # (r,k) -> (r, ij, k), broadcast ij
                        return bcv(t2.rearrange("p (r k) -> p r k", k=K0).unsqueeze(2),
                                   (P, rc, K2, K0))

                    mindet = ppool.tile([P, PP], F32, tag="mindet")
                    mindet_rij = mindet.rearrange("p (r ij) -> p r ij", ij=K2)
                    HIJ = K2 // 2
                    for h in range(2):
                        ij0 = h * HIJ

                        def Uvh(t):
                            return bcv(t.rearrange("p (ij k) -> p ij k", k=K0)
                                       [:, ij0:ij0 + HIJ, :].unsqueeze(1),
                                       (P, rc, HIJ, K0))

                        def Kvh(t2):
                            return bcv(t2.rearrange("p (r k) -> p r k", k=K0)
                                       .unsqueeze(2), (P, rc, HIJ, K0))

                        sA = spool.tile([P, S // 2], F32, tag="sA")
                        sB = spool.tile([P, S // 2], F32, tag="sB")
                        sAv = sA.rearrange("p (r ij k) -> p r ij k", ij=HIJ, k=K0)
                        sBv = sB.rearrange("p (r ij k) -> p r ij k", ij=HIJ, k=K0)
                        VE.tensor_tensor(out=sAv, in0=Kvh(v0x), in1=Uvh(U1), op=OP.mult)
                        GP.tensor_tensor(out=sBv, in0=Kvh(v0y), in1=Uvh(U2), op=OP.mult)
                        VE.tensor_tensor(out=sA, in0=sA, in1=sB, op=OP.subtract)
                        GP.tensor_tensor(out=sBv, in0=Kvh(ass), in1=Uvh(U3), op=OP.mult)
                        VE.tensor_tensor(out=sA, in0=sA, in1=sB, op=OP.add)
                        VE.tensor_reduce(out=mindet_rij[:, :, ij0:ij0 + HIJ],
                                         in_=sAv, axis=AX.X, op=OP.min)

                    # ---------- stage D: pair weights; (r, ij)=(r, i, j) ----------
                    # (r,k) tiles as (r, i, j): i from k slot / j broadcast, or
                    # j from k slot / i broadcast.
                    def XI(t2):
                        return bcv(t2.rearrange("p (r k) -> p r k", k=K0).unsqueeze(3),
                                   (P, rc, K0, K0))

                    def XJ(t2):
                        return bcv(t2.rearrange("p (r k) -> p r k", k=K0).unsqueeze(2),
                                   (P, rc, K0, K0))

                    dot01 = ppool2.tile([P, PP], F32, tag="dot01")
                    pA = ppool2.tile([P, PP], F32, tag="pA")
                    pB = ppool2.tile([P, PP], F32, tag="pB")
                    w1t = ppool.tile([P, PP], F32, tag="w1t")
                    w2t = ppool.tile([P, PP], F32, tag="w2t")
                    w0t = ppool.tile([P, PP], F32, tag="w0t")
                    inv = ppool.tile([P, PP], F32, tag="inv")
                    dot01v = dot01.rearrange("p (r i j) -> p r i j", i=K0, j=K0)
                    pAv = pA.rearrange("p (r i j) -> p r i j", i=K0, j=K0)
                    pBv = pB.rearrange("p (r i j) -> p r i j", i=K0, j=K0)

                    VE.tensor_tensor(out=dot01v, in0=XI(v0x), in1=XJ(v0x), op=OP.mult)
                    GP.tensor_tensor(out=pAv, in0=XI(v0y), in1=XJ(v0y), op=OP.mult)
                    VE.tensor_tensor(out=dot01, in0=dot01, in1=pA, op=OP.add)
                    GP.tensor_tensor(out=pAv, in0=XI(d00), in1=XJ(d00), op=OP.mult)
                    VE.tensor_tensor(out=pB, in0=dot01, in1=dot01, op=OP.mult)
                    VE.tensor_tensor(out=pA, in0=pA, in1=pB, op=OP.subtract)  # denom
                    VE.reciprocal(out=inv, in_=pA)
                    # newton refine, then clamp to +-BIG (kills inf -> no NaNs later)
                    GP.tensor_tensor(out=pB, in0=pA, in1=inv, op=OP.mult)
                    SC.activation(out=pB, in_=pB, func=AF.Copy, bias=2.0, scale=-1.0)
                    VE.tensor_tensor(out=inv, in0=inv, in1=pB, op=OP.mult)
                    VE.tensor_scalar(out=inv, in0=inv, scalar1=BIG, scalar2=-BIG,
                                     op0=OP.min, op1=OP.max)

                    VE.tensor_tensor(out=pAv, in0=XJ(d00), in1=XI(d02), op=OP.mult)
                    GP.tensor_tensor(out=pBv, in0=dot01v, in1=XJ(d02), op=OP.mult)
                    VE.tensor_tensor(out=w2t, in0=pA, in1=pB, op=OP.subtract)
                    VE.tensor_tensor(out=w2t, in0=w2t, in1=inv, op=OP.mult)
                    GP.tensor_tensor(out=pAv, in0=XI(d00), in1=XJ(d02), op=OP.mult)
                    VE.tensor_tensor(out=pBv, in0=dot01v, in1=XI(d02), op=OP.mult)
                    VE.tensor_tensor(out=w1t, in0=pA, in1=pB, op=OP.subtract)
                    VE.tensor_tensor(out=w1t, in0=w1t, in1=inv, op=OP.mult)
                    VE.tensor_tensor(out=pA, in0=w2t, in1=w1t, op=OP.add)
                    SC.activation(out=w0t, in_=pA, func=AF.Copy, bias=1.0, scale=-1.0)

                    wm = ppool.tile([P, PP], F32, tag="wm")
                    VE.tensor_tensor(out=wm, in0=w1t, in1=w2t, op=OP.min)
                    VE.tensor_tensor(out=wm, in0=wm, in1=w0t, op=OP.min)
                    sq = ppool.tile([P, PP], F32, tag="sq")
                    sr = ppool.tile([P, PP], F32, tag="sr")
                    SC.activation(out=sr, in_=w0t, func=AF.Square)
                    SC.activation(out=sq, in_=w1t, func=AF.Square)
                    VE.tensor_tensor(out=sr, in0=sr, in1=sq, op=OP.max)
                    SC.activation(out=sq, in_=w2t, func=AF.Square)
                    VE.tensor_tensor(out=sr, in0=sr, in1=sq, op=OP.max)

                    # valid = (wm>0)*(mindet>=0)*dgt_i*dgt_j*neq  (fused pairs)
                    valid = ppool.tile([P, PP], F32, tag="valid")
                    validv = valid.rearrange("p (r i j) -> p r i j", i=K0, j=K0)
                    VE.scalar_tensor_tensor(out=validv, in0=wm.rearrange(
                        "p (r i j) -> p r i j", i=K0, j=K0), scalar=0.0,
                        in1=XI(dgt), op0=OP.is_gt, op1=OP.mult)
                    GP.tensor_tensor(out=pBv, in0=XJ(dgt), in1=bcv(
                        NEQ.rearrange("p (i j) -> p i j", j=K0).unsqueeze(1),
                        (P, rc, K0, K0)), op=OP.mult)
                    VE.scalar_tensor_tensor(out=pA, in0=mindet, scalar=0.0,
                                            in1=pB, op0=OP.is_ge, op1=OP.mult)
                    VE.tensor_tensor(out=valid, in0=valid, in1=pA, op=OP.mult)

                    # score = max(sr, BIG*(1-valid)); arithmetic, NaN-free
                    score = ppool.tile([P, PP], F32, tag="score")
                    VE.tensor_scalar(out=score, in0=valid, scalar1=-BIG, scalar2=BIG,
                                     op0=OP.mult, op1=OP.add)
                    VE.tensor_tensor(out=score, in0=score, in1=sr, op=OP.max)
                    scorev = score.rearrange("p (r q) -> p r q", q=K2)
                    smin = opool.tile([P, rc], F32, tag="smin")
                    VE.tensor_reduce(out=smin, in_=scorev, axis=AX.X, op=OP.min)
                    smin_q = bcv(smin.unsqueeze(2), (P, rc, K2))
                    eqm = ppool.tile([P, PP], F32, tag="eqm")
                    eqmv = eqm.rearrange("p (r q) -> p r q", q=K2)
                    VE.tensor_tensor(out=eqmv, in0=scorev, in1=smin_q, op=OP.is_equal)
                    # pidt = iota64 where eqm else BIG  ->  first-index argmin
                    pidt = ppool.tile([P, PP], F32, tag="score")
                    pidtv = pidt.rearrange("p (r q) -> p r q", q=K2)
                    VE.tensor_scalar(out=pidt, in0=eqm, scalar1=-BIG, scalar2=BIG,
                                     op0=OP.mult, op1=OP.add)
                    i64_q = bcv(IOTA64.unsqueeze(1), (P, rc, K2))
                    VE.tensor_tensor(out=pidtv, in0=pidtv, in1=i64_q, op=OP.max)
                    pidx = opool.tile([P, rc], F32, tag="pidx")
                    VE.tensor_reduce(out=pidx, in_=pidtv, axis=AX.X, op=OP.min)
                    pidx_q = bcv(pidx.unsqueeze(2), (P, rc, K2))
                    oh = ppool.tile([P, PP], F32, tag="eqm")
                    ohv_ = oh.rearrange("p (r q) -> p r q", q=K2)
                    VE.tensor_tensor(out=ohv_, in0=i64_q, in1=pidx_q, op=OP.is_equal)

                    # gathers: weights (NaN-free now), pair indices
                    g0 = ppool2.tile([P, PP], F32, tag="dot01")
                    g0v = g0.rearrange("p (r q) -> p r q", q=K2)
                    w0sel = opool.tile([P, rc], F32, tag="w0sel")
                    w1sel = opool.tile([P, rc], F32, tag="w1sel")
                    w2sel = opool.tile([P, rc], F32, tag="w2sel")
                    i_f = opool.tile([P, rc], F32, tag="i_f")
                    j_f = opool.tile([P, rc], F32, tag="j_f")
                    GP.tensor_tensor(out=g0, in0=oh, in1=w0t, op=OP.mult)
                    VE.tensor_reduce(out=w0sel, in_=g0v, axis=AX.X, op=OP.add)
                    GP.tensor_tensor(out=g0, in0=oh, in1=w2t, op=OP.mult)
                    VE.tensor_reduce(out=w2sel, in_=g0v, axis=AX.X, op=OP.add)
                    GP.tensor_tensor(out=g0, in0=oh, in1=w1t, op=OP.mult)
                    VE.tensor_reduce(out=w1sel, in_=g0v, axis=AX.X, op=OP.add)
                    i64I = bcv(IOTAI.unsqueeze(1), (P, rc, K2))
                    i64J = bcv(IOTAJ.unsqueeze(1), (P, rc, K2))
                    GP.tensor_tensor(out=g0v, in0=ohv_, in1=i64I, op=OP.mult)
                    VE.tensor_reduce(out=i_f, in_=g0v, axis=AX.X, op=OP.add)
                    GP.tensor_tensor(out=g0v, in0=ohv_, in1=i64J, op=OP.mult)
                    VE.tensor_reduce(out=j_f, in_=g0v, axis=AX.X, op=OP.add)

                    # fallback: all candidates invalid -> w=0, idx=[c, o1, o1]
                    fb = opool.tile([P, rc], F32, tag="fb")
                    nfb = opool.tile([P, rc], F32, tag="nfb")
                    VE.tensor_scalar(out=fb, in0=smin, scalar1=1.0e38, scalar2=None,
                                     op0=OP.is_ge)
                    VE.tensor_scalar(out=nfb, in0=fb, scalar1=-1.0, scalar2=1.0,
                                     op0=OP.mult, op1=OP.add)
                    VE.tensor_tensor(out=w0sel, in0=w0sel, in1=nfb, op=OP.mult)
                    VE.tensor_tensor(out=w1sel, in0=w1sel, in1=nfb, op=OP.mult)
                    VE.tensor_tensor(out=w2sel, in0=w2sel, in1=nfb, op=OP.mult)
                    VE.copy_predicated(out=i_f, mask=fb.bitcast(I32), data=o1_f)
                    VE.copy_predicated(out=j_f, mask=fb.bitcast(I32), data=o1_f)

                    # ---------- outputs ----------
                    wout = opool.tile([P, rc * 3], F32, tag="wout")
                    iout = opool.tile([P, rc * 3], F32, tag="iout")
                    woutv = wout.rearrange("p (r c) -> p r c", c=3)
                    ioutv = iout.rearrange("p (r c) -> p r c", c=3)
                    SC.copy(out=woutv[:, :, 0], in_=w0sel)
                    SC.copy(out=woutv[:, :, 1], in_=w2sel)
                    SC.copy(out=woutv[:, :, 2], in_=w1sel)
                    SC.copy(out=ioutv[:, :, 0], in_=c_f)
                    SC.copy(out=ioutv[:, :, 1], in_=i_f)
                    SC.copy(out=ioutv[:, :, 2], in_=j_f)
                    nc.sync.dma_start(outw_d[v0_:v1_, r0:r1, :], woutv)
                    nc.sync.dma_start(outi_d[v0_:v1_, r0:r1, :], ioutv)

    nc.compile()
    return nc


def make_consts(rc=RC):
    eye = np.eye(K0, dtype=bool)
    neq = (~eye).astype(np.float32).reshape(1, K2)
    iota8 = np.arange(K0, dtype=np.float32).reshape(1, K0)
    iotaI = (np.arange(K2) // K0).astype(np.float32).reshape(1, K2)
    iotaJ = (np.arange(K2) % K0).astype(np.float32).reshape(1, K2)
    iota64 = np.arange(K2, dtype=np.float32).reshape(1, K2)
    return {"neq": neq, "iota8": iota8, "iotaI": iotaI, "iotaJ": iotaJ,
            "iota64": iota64}


def make_in_maps(template, projections):
    template = np.ascontiguousarray(np.asarray(template, np.float32))
    projections = np.ascontiguousarray(np.asarray(projections, np.float32))
    consts = make_consts()
    tmplT = np.stack([template[..., 0].reshape(-1), template[..., 1].reshape(-1)])
    px_all = np.ascontiguousarray(projections[..., 0])
    py_all = np.ascontiguousarray(projections[..., 1])
    in_maps = []
    for c in range(N_CORES):
        pxc = px_all[c * VS:(c + 1) * VS]
        pyc = py_all[c * VS:(c + 1) * VS]
        pad = VSP - VS
        pxc = np.concatenate([pxc, np.broadcast_to(pxc[:1], (pad, K0))], 0)
        pyc = np.concatenate([pyc, np.broadcast_to(pyc[:1], (pad, K0))], 0)
        m = {"px": np.ascontiguousarray(pxc), "py": np.ascontiguousarray(pyc),
             "tmpl": tmplT}
        m.update(consts)
        in_maps.append(m)
    return in_maps


_NC_CACHE = {}


def kernel(template, projections, _want_time=False):
    from concourse.bass_utils import run_bass_kernel_spmd
    if "nc" not in _NC_CACHE:
        _NC_CACHE["nc"] = build_nc()
    nc = _NC_CACHE["nc"]
    in_maps = make_in_maps(template, projections)
    res = run_bass_kernel_spmd(nc, in_maps, core_ids=list(range(N_CORES)))
    ws, idxs = [], []
    for c in range(N_CORES):
        out = res.results[c]
        ws.append(out["outw"][:VS].reshape(VS, R, A, 3))
        idxs.append(out["outi"][:VS].reshape(VS, R, A, 3))
    w = np.concatenate(ws, 0).astype(np.float32)
    idx = np.rint(np.concatenate(idxs, 0)).astype(np.int32)
    if _want_time:
        return (w, idx), res
    return w, idx
